# revision 1
# baseline (speedup 1.0000x reference)
"""Masked phase-locking value (PLV) kernel for Trainium2, 8 NeuronCores.

Math: out[b] = |sum_ij M_ij * exp(i*(a_bi - b_bj))| / max(sum(M), 1)
    real_b = sum_ij M_ij (cos a_bi cos b_bj + sin a_bi sin b_bj)
    imag_b = sum_ij M_ij (sin a_bi cos b_bj - cos a_bi sin b_bj)

Device decomposition (per core, Na sharded 8 ways -> 1024 rows each):
    acc[m, j] = sum_i W[i, m] * mask[i, j]     (TensorE; W = [ca^T | sa^T], m = 2B = 128)
    racc[m]   = sum_j acc[m, j] * CS[m, j]     (DVE mult, ACT accumulate; CS = [cb; sb])
    qacc[m]   = sum_j acc[m, j] * SW[m, j]     (SW = [sb; cb], partition-swap of CS)
real_b = sum_cores racc[b] + racc[64+b]; imag_b = sum_cores qacc[64+b] - qacc[b].
All bilinear in mask rows, so Na-shard partials just add; host does the tiny
fold + |z| / sum(M).

dtypes: mask is 0/1 -> exact in fp8e4 (1 byte, halves HBM traffic, full PE rate);
weights/CS fp16 (PE full rate); SW in fp8 (the imag side is an incoherent sum,
tiny vs the coherent real part, so fp8 there costs ~1e-5 extra error);
PSUM/epilogue fp32. End-to-end rel err ~2e-5.
Column groups are sized small-big-small: a small first group starts the PE
early, small last groups shorten the end-of-stream epilogue tail. Trig rides
the scalar HWDGE ring (doesn't queue behind masks); a PE warm-up burst during
the DMA lead-in defeats the HAM cold-clock penalty.
"""

import numpy as np

import concourse.bass as bass
import concourse.tile as tile
from concourse import bacc, mybir
from concourse.bass_utils import run_bass_kernel_spmd

B = 64
NA = 8192
NB = 8192
NCORES = 8
NASH = NA // NCORES          # mask rows per core
KCH = NASH // 128            # contraction chunks of 128 rows
NCH = 512                    # output columns per PSUM bank / matmul

# column group widths: small first (early PE start), small last (short tail)
GWS = [512, 1024, 1024, 1024, 1024, 1024, 1024, 512, 512, 256, 256]
assert sum(GWS) == NB and all(w % 256 == 0 for w in GWS)
NG = len(GWS)
GOFF = [sum(GWS[:i]) for i in range(NG)]

# trig upload pieces (scalar ring): first small so group 0's epilogue isn't gated
TP = [1024, 2048, 2560, 2560]
assert sum(TP) == NB
TPOFF = [sum(TP[:i]) for i in range(len(TP))]

F8 = mybir.dt.float8e4
F16 = mybir.dt.float16
F32 = mybir.dt.float32


def build_program() -> bass.Bass:
    nc = bacc.Bacc("TRN2")
    # concatenated per-group blocks, each contiguous [128, KCH, gw]
    mask_d = nc.dram_tensor("mask", [128 * KCH * NB], F8, kind="ExternalInput")
    w_d = nc.dram_tensor("w", [128, KCH, 2 * B], F16, kind="ExternalInput")
    cs_d = nc.dram_tensor("cs", [128, NB], F16, kind="ExternalInput")
    sw_d = nc.dram_tensor("sw", [128, NB], F8, kind="ExternalInput")
    out_d = nc.dram_tensor("out", [128, 2 * NG], F32, kind="ExternalOutput")

    copy_f = mybir.ActivationFunctionType.Copy

    with tile.TileContext(nc) as tc:
        with (
            tc.tile_pool(name="consts", bufs=1) as consts,
            tc.tile_pool(name="masks", bufs=NG) as masks,
            tc.tile_pool(name="scratch", bufs=3) as scratch,
            tc.tile_pool(name="junk", bufs=2) as junkp,
            tc.tile_pool(name="psum", bufs=3, space="PSUM") as psum_pool,
            tc.tile_pool(name="wups", bufs=1, space="PSUM") as wu_pool,
        ):
            w_sb = consts.tile([128, KCH, 2 * B], F16)
            nc.sync.dma_start(out=w_sb[:], in_=w_d[:])
            cs_sb = consts.tile([128, NB], F16)
            sw_sb = consts.tile([128, NB], F8)
            racc = consts.tile([128, 2 * NG], F32)

            # PE warm-up while the first mask group is in flight (HAM ramp)
            wu_ps = wu_pool.tile([128, 2 * B], F32)
            for r in range(16):
                nc.tensor.matmul(
                    out=wu_ps[:],
                    lhsT=w_sb[:, 0, :],
                    rhs=w_sb[:, 1, :],
                    start=(r == 0),
                    stop=(r == 15),
                )

            tp_emitted = 0
            for g in range(NG):
                off, gw = GOFF[g], GWS[g]
                gsl = slice(off, off + gw)
                mt = masks.tile([128, KCH, gw], F8, tag="mask")
                blk = 128 * KCH
                src = mask_d[off * blk : (off + gw) * blk].rearrange(
                    "(p k c) -> p k c", p=128, k=KCH
                )
                nc.sync.dma_start(out=mt[:], in_=src)
                # trig pieces on the scalar HWDGE ring, paced ahead of use
                while tp_emitted < len(TP) and TPOFF[tp_emitted] < off + gw:
                    tsl = slice(TPOFF[tp_emitted], TPOFF[tp_emitted] + TP[tp_emitted])
                    nc.scalar.dma_start(out=cs_sb[:, tsl], in_=cs_d[:, tsl])
                    nc.scalar.dma_start(out=sw_sb[:, tsl], in_=sw_d[:, tsl])
                    tp_emitted += 1

                ps = psum_pool.tile([128, gw], F32, tag="psum")
                for j0 in range(0, gw, NCH):
                    jsl = slice(j0, min(j0 + NCH, gw))
                    for k in range(KCH):
                        nc.tensor.matmul(
                            out=ps[:, jsl],
                            lhsT=w_sb[:, k, :],
                            rhs=mt[:, k, jsl],
                            start=(k == 0),
                            stop=(k == KCH - 1),
                        )
                rcol = g if g < 8 else 16 + (g - 8)
                qcol = 8 + g if g < 8 else 16 + (NG - 8) + (g - 8)
                pr = scratch.tile([128, gw], F32, tag="pr")
                nc.vector.tensor_mul(out=pr[:], in0=ps[:], in1=cs_sb[:, gsl])
                jr = junkp.tile([128, gw], F32, tag="junk")
                nc.scalar.activation(
                    out=jr[:], in_=pr[:], func=copy_f,
                    accum_out=racc[:, rcol : rcol + 1],
                )
                pi = scratch.tile([128, gw], F32, tag="pr")
                nc.vector.tensor_mul(out=pi[:], in0=ps[:], in1=sw_sb[:, gsl])
                ji = junkp.tile([128, gw], F32, tag="junk")
                nc.scalar.activation(
                    out=ji[:], in_=pi[:], func=copy_f,
                    accum_out=racc[:, qcol : qcol + 1],
                )
                if g == 7:
                    # groups 0-7 partials fly out while tail groups finish
                    nc.sync.dma_start(out=out_d[:, :16], in_=racc[:, :16])

            nc.sync.dma_start(out=out_d[:, 16:], in_=racc[:, 16:])
    nc.finalize()
    return nc


def prep_inputs(phases_a, phases_b, coupling_mask):
    pa = np.asarray(phases_a, dtype=np.float32)
    pb = np.asarray(phases_b, dtype=np.float32)
    ca, sa = np.cos(pa), np.sin(pa)
    cb, sb = np.cos(pb), np.sin(pb)
    cs = np.concatenate([cb, sb], axis=0).astype(np.float16)
    sw = np.concatenate([sb, cb], axis=0).astype(mybir.dt.np(F8))

    f8np = mybir.dt.np(F8)
    one_byte = np.array([1.0], f8np).view(np.uint8)[0]
    mask_u8 = (np.asarray(coupling_mask) != 0).astype(np.uint8) * one_byte

    in_maps = []
    for c in range(NCORES):
        rows = slice(c * NASH, (c + 1) * NASH)
        W = np.empty((NASH, 2 * B), np.float16)
        W[:, :B] = ca[:, rows].T
        W[:, B:] = sa[:, rows].T
        # [i = k*128 + p, m] -> [p, k, m]
        w_host = np.ascontiguousarray(W.reshape(KCH, 128, 2 * B).transpose(1, 0, 2))
        # per group: contiguous [p, k, c] block; blocks concatenated flat
        mr = mask_u8[rows].reshape(KCH, 128, NB)
        blocks = [
            np.ascontiguousarray(
                mr[:, :, GOFF[g] : GOFF[g] + GWS[g]].transpose(1, 0, 2)
            ).reshape(-1)
            for g in range(NG)
        ]
        m_host = np.concatenate(blocks).view(f8np)
        in_maps.append({"mask": m_host, "w": w_host, "cs": cs, "sw": sw})
    return in_maps


def combine(outs, coupling_mask):
    o = np.stack(outs).astype(np.float64)  # [NCORES, 128, 2*NG]
    nt = NG - 8
    r = o[:, :, :8].sum(axis=2) + o[:, :, 16 : 16 + nt].sum(axis=2)
    q = o[:, :, 8:16].sum(axis=2) + o[:, :, 16 + nt :].sum(axis=2)
    real = (r[:, :B] + r[:, B:]).sum(axis=0)
    imag = (q[:, B:] - q[:, :B]).sum(axis=0)
    n_pairs = max(float(np.asarray(coupling_mask).sum()), 1.0)
    return (np.sqrt(real * real + imag * imag) / n_pairs).astype(np.float32)


_prog_cache: list = []


def kernel(phases_a, phases_b, coupling_mask):
    in_maps = prep_inputs(phases_a, phases_b, coupling_mask)
    if not _prog_cache:
        _prog_cache.append(build_program())
    res = run_bass_kernel_spmd(_prog_cache[0], in_maps, core_ids=list(range(NCORES)))
    return combine([r["out"] for r in res.results], coupling_mask)



# revision 5
# speedup vs baseline: 1.4604x; 1.4604x over previous
"""Masked phase-locking value (PLV) kernel for Trainium2, 8 NeuronCores.

Math: out[b] = |sum_ij M_ij * exp(i*(a_bi - b_bj))| / max(sum(M), 1)

Device decomposition (per core, Na sharded 8 ways -> 1024 i-columns each):
    Z[c, i] = sum_j cs2[j, c] * M[i, j]        (TensorE, fp8 DoubleRow)
with cs2 = [cb; sb] stacked along c (c = 2B = 128) as the STATIONARY
operand and the transposed mask streaming through. The whole Nb=8192
contraction accumulates in one PSUM bank per 512-wide i-block, so the
epilogue shrinks to the sharded dim:
    racc[c] = sum_i Z[c, i] * WA[c, i]          (DVE tensor_tensor_reduce)
    qacc[c] = sum_i Z[c, i] * WS[c, i]
with WA = [ca; sa], WS = [sa; -ca] (sign folded in on host), giving
real_b = sum_cores racc[b] + racc[64+b], imag_b likewise from qacc.

dtypes: mask 0/1 and cs2 in fp8e4 -> DoubleRow double-pumped matmul
(2 contraction rows/cycle); WA/WS fp16; PSUM/reduce fp32. Host-emulated
end-to-end rel err 2.3e-3 (gate 2e-2).

The kernel is HBM-DMA-bound (~9.5 MB/core at ~360 GB/s): mask groups are
sized small-big-big-small per i-block so the PE starts early and the tail
epilogue is short; cs2 rides the scalar HWDGE ring in paced pieces; a PE
warm-up burst during the DMA lead-in defeats the HAM cold-clock penalty.
"""

import numpy as np

import concourse.bass as bass
import concourse.tile as tile
from concourse import bacc, mybir
from concourse.bass_utils import run_bass_kernel_spmd

B = 64
NA = 8192
NB = 8192
NCORES = 8
NASH = NA // NCORES          # i-columns per core
JC = NB // 256               # 32 DoubleRow j-chunks of 256
NIB = 2                      # i-blocks of 512 (one PSUM bank each)
IBW = NASH // NIB

# j-chunks per mask DMA group, per i-block: small first (early PE start),
# small last (short tail before the final epilogue)
GJC = [[4, 8, 12, 8], [8, 12, 8, 4]]
assert all(sum(g) == JC for g in GJC)

NWU = 18                     # PE warm-up matmuls during DMA lead-in

F8 = mybir.dt.float8e4
F16 = mybir.dt.float16
F32 = mybir.dt.float32
DR = mybir.MatmulPerfMode.DoubleRow
MUL = mybir.AluOpType.mult
ADD = mybir.AluOpType.add


def build_program() -> bass.Bass:
    nc = bacc.Bacc("TRN2")
    # mask: concatenated group blocks, each contiguous [128, gjc, 2, 512]
    mask_d = nc.dram_tensor("mask", [128 * JC * 2 * IBW * NIB], F8,
                            kind="ExternalInput")
    # cs2: 4 contiguous pieces of [128, 8, 2, 128]
    cs2_d = nc.dram_tensor("cs2", [128 * JC * 2 * 128], F8, kind="ExternalInput")
    wv_d = nc.dram_tensor("wv", [128, 2, NASH], F16, kind="ExternalInput")
    out_d = nc.dram_tensor("out", [128, 2 * NIB], F32, kind="ExternalOutput")

    with tile.TileContext(nc) as tc:
        with (
            tc.tile_pool(name="consts", bufs=1) as consts,
            tc.tile_pool(name="masks", bufs=4) as masks,
            tc.tile_pool(name="scratch", bufs=2) as scratch,
            tc.tile_pool(name="junk", bufs=2) as junkp,
            tc.tile_pool(name="zpsum", bufs=NIB, space="PSUM") as zpool,
            tc.tile_pool(name="wups", bufs=1, space="PSUM") as wu_pool,
        ):
            cs2_sb = consts.tile([128, JC, 2, 128], F8)
            psz = 128 * 8 * 2 * 128
            for piece in range(4):
                src = cs2_d[piece * psz : (piece + 1) * psz].rearrange(
                    "(p j t c) -> p j t c", p=128, j=8, t=2
                )
                nc.scalar.dma_start(out=cs2_sb[:, 8 * piece : 8 * piece + 8], in_=src)
            wv_sb = consts.tile([128, 2, NASH], F16)
            nc.scalar.dma_start(out=wv_sb[:], in_=wv_d[:])

            # PE warm-up on a memset tile while the first mask group is in
            # flight (HAM clock ramp)
            wu8 = consts.tile([128, 128], F8)
            nc.vector.memset(wu8[:], 1.0)
            wu_ps = wu_pool.tile([128, 128], F32)
            for r in range(NWU):
                nc.tensor.matmul(
                    out=wu_ps[:], lhsT=wu8[:], rhs=wu8[:],
                    start=(r == 0), stop=(r == NWU - 1),
                )

            raccs = []
            off = 0
            for ib in range(NIB):
                z = zpool.tile([128, IBW], F32, tag=f"z{ib}")
                jc = 0
                for gjc in GJC[ib]:
                    sz = 128 * gjc * 2 * IBW
                    mt = masks.tile([128, gjc, 2, IBW], F8, tag="mask")
                    src = mask_d[off : off + sz].rearrange(
                        "(p k t i) -> p k t i", p=128, k=gjc, t=2
                    )
                    nc.sync.dma_start(out=mt[:], in_=src)
                    off += sz
                    for k in range(gjc):
                        nc.tensor.matmul(
                            out=z[:],
                            lhsT=cs2_sb[:, jc],
                            rhs=mt[:, k],
                            start=(jc == 0),
                            stop=(jc == JC - 1),
                            perf_mode=DR,
                        )
                        jc += 1
                isl = slice(ib * IBW, (ib + 1) * IBW)
                racc = consts.tile([128, 2], F32, name=f"racc{ib}")
                for q in range(2):  # 0: real-side (WA), 1: imag-side (WS)
                    pr = scratch.tile([128, IBW], F32, tag="prod")
                    nc.vector.tensor_mul(out=pr[:], in0=z[:], in1=wv_sb[:, q, isl])
                    jr = junkp.tile([128, IBW], F32, tag="junk")
                    nc.scalar.activation(
                        out=jr[:], in_=pr[:],
                        func=mybir.ActivationFunctionType.Copy,
                        accum_out=racc[:, q : q + 1],
                    )
                nc.scalar.dma_start(out=out_d[:, 2 * ib : 2 * ib + 2], in_=racc[:])
                raccs.append(racc)
    nc.finalize()
    return nc


def prep_inputs(phases_a, phases_b, coupling_mask):
    pa = np.asarray(phases_a, dtype=np.float32)
    pb = np.asarray(phases_b, dtype=np.float32)
    ca, sa = np.cos(pa), np.sin(pa)
    cb, sb = np.cos(pb), np.sin(pb)
    f8np = mybir.dt.np(F8)

    # cs2 pieces: [piece, p, jck, t, c] with j = (8*piece + jck)*256 + t*128 + p
    CS2 = np.concatenate([cb, sb], axis=0).astype(f8np)     # [c, j]
    cs2_host = np.ascontiguousarray(
        CS2.T.reshape(4, 8, 2, 128, 128).transpose(0, 3, 1, 2, 4)
    ).reshape(-1)

    one_byte = np.array([1.0], f8np).view(np.uint8)[0]
    mask_u8 = (np.asarray(coupling_mask) != 0).astype(np.uint8) * one_byte
    MT = np.ascontiguousarray(mask_u8.T)                    # [j, i]

    WA = np.concatenate([ca, sa], axis=0)                   # [c, i_full]
    WS = np.concatenate([sa, -ca], axis=0)

    in_maps = []
    for c in range(NCORES):
        isl = slice(c * NASH, (c + 1) * NASH)
        # [jc, t, p, ib, ii] -> [ib, jc, p, t, ii]
        v = MT[:, isl].reshape(JC, 2, 128, NIB, IBW).transpose(3, 0, 2, 1, 4)
        blocks = []
        for ib in range(NIB):
            jc = 0
            for gjc in GJC[ib]:
                blk = v[ib, jc : jc + gjc].transpose(1, 0, 2, 3)  # [p, k, t, ii]
                blocks.append(np.ascontiguousarray(blk).reshape(-1))
                jc += gjc
        m_host = np.concatenate(blocks).view(f8np)
        wv = np.stack([WA[:, isl], WS[:, isl]], axis=1).astype(np.float16)
        in_maps.append({"mask": m_host, "cs2": cs2_host, "wv": wv})
    return in_maps


def combine(outs, coupling_mask):
    o = np.stack(outs).astype(np.float64)   # [NCORES, 128, 2*NIB]
    r = o[:, :, 0::2].sum(axis=2)           # [NCORES, 128]
    q = o[:, :, 1::2].sum(axis=2)
    real = (r[:, :B] + r[:, B:]).sum(axis=0)
    imag = (q[:, :B] + q[:, B:]).sum(axis=0)
    n_pairs = max(float(np.asarray(coupling_mask).sum()), 1.0)
    return (np.sqrt(real * real + imag * imag) / n_pairs).astype(np.float32)


_prog_cache: list = []


def kernel(phases_a, phases_b, coupling_mask):
    in_maps = prep_inputs(phases_a, phases_b, coupling_mask)
    if not _prog_cache:
        _prog_cache.append(build_program())
    res = run_bass_kernel_spmd(_prog_cache[0], in_maps, core_ids=list(range(NCORES)))
    return combine([r["out"] for r in res.results], coupling_mask)
